# revision 4
# baseline (speedup 1.0000x reference)
# Contrastive (NT-Xent style) loss kernel for 8 Trainium2 NeuronCores.
#
# Math: with z = concat(z_i, z_j)  (N=8192 rows, D=128), zn = row-normalized z,
# sim = (zn @ zn.T)/TEMP, the reference loss reduces exactly to
#   loss = (1/N) * sum_r [ log( sum_{c != r} exp(sim[r,c]) ) - sim[r, (r+B) % N] ]
# (verified bit-for-bit against the reference's mask/gather formulation).
#
# Sharding: data-parallel over rows. Core m receives z rolled by -1024*m rows,
# so every core runs the IDENTICAL program on "its" rows 0..1023 (rotation
# makes the diagonal / positive-pair columns core-independent: the positive
# partner of rotated row r is rotated column (r + 4096) % 8192 for all cores).
# Each core computes, for its 8 row-tiles of 128 rows:
#   - full-row sums of exp(2*dot) via TensorE matmul (bf16 in, fp32 acc)
#     + ScalarE exp with accumulate
#   - the raw diagonal dot and positive-pair dot (extracted from PSUM)
#   - per-row loss term log(rowsum - exp(2*diag)) - 2*pos
# The host sums the 8x(128x8) per-row terms and divides by N.

import numpy as np

B = 4096
D = 128
N = 2 * B
TEMP = 0.5
NCORES = 8
RPC = N // NCORES          # rows per core = 1024
NT = N // 128              # 64 column tiles of 128
RT = RPC // 128            # 8 row tiles per core
# PSUM groups per row-tile: 5x1536 + 1x512 = 8192 columns
GROUPS = [(0, 1536), (1536, 1536), (3072, 1536), (4608, 1536), (6144, 1536), (7680, 512)]

_CACHE = {}


def _build():
    import concourse.bass as bass
    import concourse.bacc as bacc
    import concourse.tile as tile
    from concourse import mybir
    import ml_dtypes

    f32 = mybir.dt.float32
    bf16 = mybir.dt.bfloat16
    Alu = mybir.AluOpType
    Act = mybir.ActivationFunctionType

    nc = bacc.Bacc(
        "TRN2",
        target_bir_lowering=False,
        debug=False,
        enable_asserts=False,
        num_devices=NCORES,
    )
    z_d = nc.dram_tensor("z", [N, D], f32, kind="ExternalInput").ap()
    out_d = nc.dram_tensor("out", [128, RT], f32, kind="ExternalOutput").ap()
    identb_d = nc.inline_tensor(np.eye(128, dtype=ml_dtypes.bfloat16), name="identb").ap()
    identf_d = nc.inline_tensor(np.eye(128, dtype=np.float32), name="identf").ap()

    zv = z_d.rearrange("(t p) d -> t p d", p=128)  # (64, 128, 128)

    with tile.TileContext(nc) as tc:
        with (
            tc.tile_pool(name="persist", bufs=1) as P,
            tc.tile_pool(name="work", bufs=3) as W,
            tc.tile_pool(name="grp", bufs=2, space="PSUM") as G,
            tc.tile_pool(name="tp", bufs=2, space="PSUM") as TP,
        ):
            zb = P.tile([128, NT, D], f32)      # raw z, row-major tiles
            znT = P.tile([128, NT, 128], bf16)  # zn transposed: [d, tile, row]
            nrm2 = P.tile([128, NT], f32)
            nrm2g = P.tile([128, NT], f32)
            rnorm = P.tile([128, NT], f32)
            partials = P.tile([128, RT * len(GROUPS)], f32)
            diag = P.tile([128, RT], f32)
            pos = P.tile([128, RT], f32)
            identb = P.tile([128, 128], bf16)
            identf = P.tile([128, 128], f32)

            nc.sync.dma_start(out=identb, in_=identb_d)
            nc.sync.dma_start(out=identf, in_=identf_d)

            # ---- load + row norms ----
            for t in range(NT):
                nc.sync.dma_start(out=zb[:, t, :], in_=zv[t])
            for t in range(NT):
                sq = W.tile([128, D], f32, tag="sq")
                nc.vector.scalar_tensor_tensor(
                    out=sq,
                    in0=zb[:, t, :],
                    scalar=1.0,
                    in1=zb[:, t, :],
                    op0=Alu.mult,
                    op1=Alu.mult,
                    accum_out=nrm2[:, t : t + 1],
                )
            # rnorm = exp(-0.5 * ln(max(nrm2, tiny)))  (Ln+Exp share one ACT table set)
            for b4 in range(4):
                sl = slice(b4 * 16, (b4 + 1) * 16)
                nc.vector.tensor_scalar_max(out=nrm2g[:, sl], in0=nrm2[:, sl], scalar1=1e-16)
                lnr = W.tile([128, 16], f32, tag="lnr")
                nc.scalar.activation(out=lnr, in_=nrm2g[:, sl], func=Act.Ln)
                nc.scalar.activation(out=rnorm[:, sl], in_=lnr, func=Act.Exp, scale=-0.5)

            # ---- normalize (bf16) + transpose ----
            for t in range(NT):
                znb = W.tile([128, D], bf16, tag="znb")
                nc.vector.tensor_scalar_mul(out=znb, in0=zb[:, t, :], scalar1=rnorm[:, t : t + 1])
                pt = TP.tile([128, 128], bf16, tag="tp")
                nc.tensor.transpose(out=pt, in_=znb, identity=identb)
                nc.vector.tensor_copy(out=znT[:, t, :], in_=pt)

            # ---- main loop: sim row-tiles -> exp row sums (+ diag/pos taps) ----
            for rt in range(RT):
                wt = znT[:, rt, :]  # (128,128) bf16 stationary: rows rt*128..+128
                for gi, (goff, gw) in enumerate(GROUPS):
                    grp = G.tile([128, 1536], f32, tag="grp")
                    for k in range(gw // 512):
                        c0 = goff + k * 512
                        nc.tensor.matmul(
                            grp[:, k * 512 : (k + 1) * 512],
                            lhsT=wt,
                            rhs=znT[:, c0 // 128 : c0 // 128 + 4, :],
                            start=True,
                            stop=True,
                        )
                    esc = W.tile([128, 1536], f32, tag="esc")
                    nc.scalar.activation(
                        out=esc[:, :gw],
                        in_=grp[:, :gw],
                        func=Act.Exp,
                        scale=2.0,
                        accum_out=partials[:, rt * len(GROUPS) + gi : rt * len(GROUPS) + gi + 1],
                    )
                    # diagonal dot tap (always lands in group 0)
                    if gi == 0:
                        dsc = W.tile([128, 128], f32, tag="dsc")
                        nc.vector.scalar_tensor_tensor(
                            out=dsc,
                            in0=grp[:, rt * 128 : rt * 128 + 128],
                            scalar=1.0,
                            in1=identf,
                            op0=Alu.mult,
                            op1=Alu.mult,
                            accum_out=diag[:, rt : rt + 1],
                        )
                    # positive-pair dot tap at column 4096 + rt*128
                    pcol = B + rt * 128
                    if goff <= pcol and pcol + 128 <= goff + gw:
                        poff = pcol - goff
                        psc = W.tile([128, 128], f32, tag="psc")
                        nc.vector.scalar_tensor_tensor(
                            out=psc,
                            in0=grp[:, poff : poff + 128],
                            scalar=1.0,
                            in1=identf,
                            op0=Alu.mult,
                            op1=Alu.mult,
                            accum_out=pos[:, rt : rt + 1],
                        )

            # ---- epilogue: per-row loss terms ----
            rows = P.tile([128, RT], f32)
            exp2d = P.tile([128, RT], f32)
            negsum = P.tile([128, RT], f32)
            lse = P.tile([128, RT], f32)
            lossb = P.tile([128, RT], f32)
            ng = len(GROUPS)
            for rt in range(RT):
                nc.vector.tensor_reduce(
                    out=rows[:, rt : rt + 1],
                    in_=partials[:, rt * ng : (rt + 1) * ng],
                    axis=mybir.AxisListType.X,
                    op=Alu.add,
                )
            nc.scalar.activation(out=exp2d, in_=diag, func=Act.Exp, scale=2.0)
            nc.vector.tensor_sub(negsum, rows, exp2d)
            nc.scalar.activation(out=lse, in_=negsum, func=Act.Ln)
            nc.vector.scalar_tensor_tensor(
                out=lossb,
                in0=pos,
                scalar=-2.0,
                in1=lse,
                op0=Alu.mult,
                op1=Alu.add,
            )
            nc.sync.dma_start(out=out_d, in_=lossb)

    nc.compile()
    return nc


def _get_nc():
    if "nc" not in _CACHE:
        _CACHE["nc"] = _build()
    return _CACHE["nc"]


def run(z_i: np.ndarray, z_j: np.ndarray, trace: bool = False):
    from concourse import bass_utils

    nc = _get_nc()
    z = np.concatenate(
        [np.asarray(z_i, dtype=np.float32), np.asarray(z_j, dtype=np.float32)], axis=0
    )
    in_maps = [
        {"z": np.ascontiguousarray(np.roll(z, -RPC * m, axis=0))} for m in range(NCORES)
    ]
    res = bass_utils.run_bass_kernel_spmd(
        nc, in_maps, core_ids=list(range(NCORES)), trace=trace
    )
    total = sum(r["out"].astype(np.float64).sum() for r in res.results)
    return np.array(total / N, dtype=np.float32), res


def kernel(z_i: np.ndarray, z_j: np.ndarray) -> np.ndarray:
    return run(z_i, z_j)[0]
